# revision 5
# baseline (speedup 1.0000x reference)
"""Cost-volume construction (nn_CostVolume) as a Bass/Trainium2 SPMD kernel.

Problem (hardcoded shapes):
    left_features:  (4, 32, 64, 128) f32
    right_features: (4, 32, 64, 128) f32
    max_disparity:  192  ->  D = 48
    output:         (4, 64, 48, 64, 128) f32
        out[:, :C, d]  = left
        out[:, C:, d, h, w] = right[:, :, h, w+d] if w+d < W else 0

This is a pure data-movement problem (384 MiB written from 8 MiB of input),
so the kernel is DMA-only. Sharding: disparity axis D=48 split as 6 per core
across 8 cores.

Trick: every DMA is made fully contiguous by padding. Each core receives
  - left  rows zero-padded from W=128 to PW=133 and flattened per (b,c)
  - right rows pre-shifted by the core's base disparity d0=6k, zero-padded
    to PW, flattened
Then for local disparity ld, the right-half output slab (H x PW, flat) is
exactly rext_flat[ld : ld + H*PW]: the shift crosses row boundaries into the
zero padding, which supplies the zero fill for w+d >= W, and the junk in the
padding columns is stripped on the host. One contiguous 4.4 MiB DMA per
(half, ld); 12 stores + 2 loads per core.
"""

import numpy as np

import concourse.bass as bass
from concourse import mybir
from concourse.bass_utils import run_bass_kernel_spmd

B, C, H, W = 4, 32, 64, 128
D = 48
NCORES = 8
DLOC = D // NCORES          # 6 disparities per core
PW = W + DLOC - 1           # 133: padded row width
SLAB = H * PW               # 8512 elements per (b, c, ld) output slab
SRCW = SLAB + DLOC - 1      # per-partition input width (flat shift tail)

_NC_CACHE = {}


def _build(repeat=1):
    """Build the SPMD program. repeat>1 re-runs the whole body that many
    times (loads + stores), serialized on the DMA semaphore — used only for
    steady-state benchmarking; the graded path uses repeat=1."""
    if repeat in _NC_CACHE:
        return _NC_CACHE[repeat]
    nc = bass.Bass()
    left_in = nc.declare_dram_parameter(
        "left", [B * C, SRCW], mybir.dt.float32, isOutput=False)
    rext_in = nc.declare_dram_parameter(
        "rext", [B * C, SRCW], mybir.dt.float32, isOutput=False)
    out_ext = nc.declare_dram_parameter(
        "out", [B, 2 * C, DLOC, SLAB], mybir.dt.float32, isOutput=True)

    with (
        nc.sbuf_tensor([B * C, SRCW], mybir.dt.float32) as left_t,
        nc.sbuf_tensor([B * C, SRCW], mybir.dt.float32) as rext_t,
        nc.semaphore("dma_sem") as dma_sem,
        nc.Block() as block,
    ):

        @block.sync
        def _(sync):
            per_iter = (2 + 2 * DLOC) * 16
            for r in range(repeat):
                base = per_iter * r
                if r:
                    sync.wait_ge(dma_sem, base)
                sync.dma_start(left_t[:], left_in[:]).then_inc(dma_sem, 16)
                sync.dma_start(rext_t[:], rext_in[:]).then_inc(dma_sem, 16)
                sync.wait_ge(dma_sem, base + 32)
                for ld in range(DLOC):
                    sync.dma_start(
                        out_ext[:, 0:C, ld, :], left_t[:, 0:SLAB]
                    ).then_inc(dma_sem, 16)
                    sync.dma_start(
                        out_ext[:, C:2 * C, ld, :], rext_t[:, ld:ld + SLAB]
                    ).then_inc(dma_sem, 16)
            sync.wait_ge(dma_sem, per_iter * repeat)

    _NC_CACHE[repeat] = nc
    return nc


def _host_inputs(left, right):
    """Per-core device input dicts (host-side shard prep)."""
    lf = left.reshape(B * C, H, W)
    rf = right.reshape(B * C, H, W)

    le_flat = np.zeros((B * C, SRCW), np.float32)
    le_flat[:, :SLAB] = np.concatenate(
        [lf, np.zeros((B * C, H, PW - W), np.float32)], axis=2
    ).reshape(B * C, SLAB)

    in_maps = []
    for k in range(NCORES):
        d0 = DLOC * k
        re = np.zeros((B * C, H, PW), np.float32)
        take = max(0, W - d0)
        re[:, :, :take] = rf[:, :, d0:d0 + take]
        re_flat = np.zeros((B * C, SRCW), np.float32)
        re_flat[:, :SLAB] = re.reshape(B * C, SLAB)
        in_maps.append({"left": le_flat, "rext": re_flat})
    return in_maps


def _run(in_maps, **kwargs):
    nc = _build()
    return run_bass_kernel_spmd(nc, in_maps, list(range(NCORES)), **kwargs)


def _gather(results):
    out = np.empty((B, 2 * C, D, H, W), np.float32)
    for k in range(NCORES):
        slab = results[k]["out"].reshape(B, 2 * C, DLOC, H, PW)
        out[:, :, DLOC * k:DLOC * (k + 1)] = slab[:, :, :, :, :W]
    return out


def kernel(left_features, right_features, max_disparity):
    left = np.asarray(left_features, dtype=np.float32)
    right = np.asarray(right_features, dtype=np.float32)
    assert int(np.asarray(max_disparity)) == 4 * D
    assert left.shape == (B, C, H, W) and right.shape == (B, C, H, W)

    in_maps = _host_inputs(left, right)
    res = _run(in_maps)
    return _gather(res.results)


# revision 8
# speedup vs baseline: 3.4248x; 3.4248x over previous
"""Cost-volume construction (nn_CostVolume) as a Bass/Trainium2 SPMD kernel.

Problem (hardcoded shapes):
    left_features:  (4, 32, 64, 128) f32
    right_features: (4, 32, 64, 128) f32
    max_disparity:  192  ->  D = 48
    output:         (4, 64, 48, 64, 128) f32
        out[:, :C, d]  = left
        out[:, C:, d, h, w] = right[:, :, h, w+d] if w+d < W else 0

This is a pure data-movement problem (384 MiB written from 8 MiB of input),
so the kernel is DMA-only. Sharding: disparity axis D=48 split as 6 per core
across 8 cores.

Trick: every DMA is made fully contiguous by padding. Each core receives
  - left  rows zero-padded from W=128 to PW=133 and flattened per (b,c)
  - right rows pre-shifted by the core's base disparity d0=6k, zero-padded
    to PW, flattened
Then for local disparity ld, the right-half output slab (H x PW, flat) is
exactly rext_flat[ld : ld + H*PW]: the shift crosses row boundaries into the
zero padding, which supplies the zero fill for w+d >= W, and the junk in the
padding columns is stripped on the host. One contiguous 4.4 MiB DMA per
(half, ld); 12 stores + 2 loads per core.
"""

import numpy as np

import concourse.bass as bass
from concourse import mybir
from concourse.bass_utils import run_bass_kernel_spmd

B, C, H, W = 4, 32, 64, 128
D = 48
NCORES = 8
DLOC = D // NCORES          # 6 disparities per core
PW = W + DLOC - 1           # 133: padded row width
SLAB = H * PW               # 8512 elements per (b, c, ld) output slab
SRCW = SLAB + DLOC - 1      # per-partition input width (flat shift tail)

_NC_CACHE = {}


def _build(repeat=1):
    """Build the SPMD program. repeat>1 re-runs the whole body that many
    times (loads + stores), serialized on the DMA semaphore — used only for
    steady-state benchmarking; the graded path uses repeat=1."""
    if repeat in _NC_CACHE:
        return _NC_CACHE[repeat]
    nc = bass.Bass()
    left_in = nc.declare_dram_parameter(
        "left", [B * C, SRCW], mybir.dt.float32, isOutput=False)
    rext_in = nc.declare_dram_parameter(
        "rext", [B * C, SRCW], mybir.dt.float32, isOutput=False)
    # Partition-major output layout [half, p=(b,c), ld, SLAB]: keeps every
    # store's DRAM AP 2-dim ([stride,128],[1,SLAB]), which the DGE splits
    # across all 16 SDMA engines. A (B, 2C, DLOC, SLAB) layout makes the
    # DRAM AP 3-dim and serializes descriptor processing (~27x slower
    # stores in the cost model, ~4x on HW).
    out_ext = nc.declare_dram_parameter(
        "out", [2, B * C, DLOC, SLAB], mybir.dt.float32, isOutput=True)

    with (
        nc.sbuf_tensor([B * C, SRCW], mybir.dt.float32) as left_t,
        nc.sbuf_tensor([B * C, SRCW], mybir.dt.float32) as rext_t,
        nc.semaphore("dma_sem") as dma_sem,
        nc.Block() as block,
    ):

        @block.sync
        def _(sync):
            per_iter = (2 + 2 * DLOC) * 16
            for r in range(repeat):
                base = per_iter * r
                if r:
                    sync.wait_ge(dma_sem, base)
                sync.dma_start(left_t[:], left_in[:]).then_inc(dma_sem, 16)
                sync.dma_start(rext_t[:], rext_in[:]).then_inc(dma_sem, 16)
                sync.wait_ge(dma_sem, base + 32)
                for ld in range(DLOC):
                    sync.dma_start(
                        out_ext[0, :, ld, :], left_t[:, 0:SLAB]
                    ).then_inc(dma_sem, 16)
                    sync.dma_start(
                        out_ext[1, :, ld, :], rext_t[:, ld:ld + SLAB]
                    ).then_inc(dma_sem, 16)
            sync.wait_ge(dma_sem, per_iter * repeat)

    _NC_CACHE[repeat] = nc
    return nc


def _host_inputs(left, right):
    """Per-core device input dicts (host-side shard prep)."""
    lf = left.reshape(B * C, H, W)
    rf = right.reshape(B * C, H, W)

    le_flat = np.zeros((B * C, SRCW), np.float32)
    le_flat[:, :SLAB] = np.concatenate(
        [lf, np.zeros((B * C, H, PW - W), np.float32)], axis=2
    ).reshape(B * C, SLAB)

    in_maps = []
    for k in range(NCORES):
        d0 = DLOC * k
        re = np.zeros((B * C, H, PW), np.float32)
        take = max(0, W - d0)
        re[:, :, :take] = rf[:, :, d0:d0 + take]
        re_flat = np.zeros((B * C, SRCW), np.float32)
        re_flat[:, :SLAB] = re.reshape(B * C, SLAB)
        in_maps.append({"left": le_flat, "rext": re_flat})
    return in_maps


def _run(in_maps, **kwargs):
    nc = _build()
    return run_bass_kernel_spmd(nc, in_maps, list(range(NCORES)), **kwargs)


def _gather(results):
    out = np.empty((B, 2 * C, D, H, W), np.float32)
    for k in range(NCORES):
        slab = results[k]["out"].reshape(2, B, C, DLOC, H, PW)
        slab = slab.transpose(1, 0, 2, 3, 4, 5).reshape(B, 2 * C, DLOC, H, PW)
        out[:, :, DLOC * k:DLOC * (k + 1)] = slab[:, :, :, :, :W]
    return out


def kernel(left_features, right_features, max_disparity):
    left = np.asarray(left_features, dtype=np.float32)
    right = np.asarray(right_features, dtype=np.float32)
    assert int(np.asarray(max_disparity)) == 4 * D
    assert left.shape == (B, C, H, W) and right.shape == (B, C, H, W)

    in_maps = _host_inputs(left, right)
    res = _run(in_maps)
    return _gather(res.results)


# revision 10
# speedup vs baseline: 3.6241x; 1.0582x over previous
"""Cost-volume construction (nn_CostVolume) as a Bass/Trainium2 SPMD kernel.

Problem (hardcoded shapes):
    left_features:  (4, 32, 64, 128) f32
    right_features: (4, 32, 64, 128) f32
    max_disparity:  192  ->  D = 48
    output:         (4, 64, 48, 64, 128) f32
        out[:, :C, d]  = left
        out[:, C:, d, h, w] = right[:, :, h, w+d] if w+d < W else 0

Pure data movement (384 MiB written from 8 MiB of input) -> DMA-only kernel.
Sharding: disparity axis D=48 split 6-per-core across 8 cores.

Key tricks:
- Right half: rows are zero-padded from W=128 to PW=133 and flattened per
  (b,c) partition, so the shifted slab for local disparity ld is exactly
  rext_flat[ld : ld + H*PW]: the shift runs across row boundaries into the
  zero padding, which provides the w+d >= W zero fill; the junk in the
  padding columns is stripped on the host. Every store is one fully
  contiguous ~4.3 MiB DMA.
- Outputs are partition-major ([p, ld, slab]) so every store's DRAM AP is
  2-dim; a channel-major layout gives 3-dim DRAM APs whose descriptors the
  DGE cannot spread across the 16 SDMA engines (measured ~3.4x slower).
- Left and right halves are fully independent chains on the two HWDGE
  rings (SP and ACT), each with its own semaphore, so the two loads and
  the 12 stores overlap.
"""

import numpy as np

import concourse.bass as bass
from concourse import mybir
from concourse.bass_utils import run_bass_kernel_spmd

B, C, H, W = 4, 32, 64, 128
D = 48
NCORES = 8
DLOC = D // NCORES          # 6 disparities per core
PW = W + DLOC - 1           # 133: padded row width (right half)
HW = H * W                  # 8192: left-half slab
SLAB = H * PW               # 8512: right-half slab
SRCW = SLAB + DLOC - 1      # right input per-partition width

_NC_CACHE = {}


def _build(repeat=1):
    """Build the SPMD program. repeat>1 re-runs the whole body that many
    times, serialized per chain on its semaphore — used only for
    steady-state benchmarking; the graded path uses repeat=1."""
    if repeat in _NC_CACHE:
        return _NC_CACHE[repeat]
    nc = bass.Bass()
    left_in = nc.declare_dram_parameter(
        "left", [B * C, HW], mybir.dt.float32, isOutput=False)
    rext_in = nc.declare_dram_parameter(
        "rext", [B * C, SRCW], mybir.dt.float32, isOutput=False)
    out_l = nc.declare_dram_parameter(
        "out_l", [B * C, DLOC, HW], mybir.dt.float32, isOutput=True)
    out_r = nc.declare_dram_parameter(
        "out_r", [B * C, DLOC, SLAB], mybir.dt.float32, isOutput=True)

    with (
        nc.sbuf_tensor([B * C, HW], mybir.dt.float32) as left_t,
        nc.sbuf_tensor([B * C, SRCW], mybir.dt.float32) as rext_t,
        nc.semaphore("sem_l") as sem_l,
        nc.semaphore("sem_r") as sem_r,
        nc.Block() as block,
    ):
        per_iter = (1 + DLOC) * 16

        @block.sync
        def _(sync):
            for r in range(repeat):
                base = per_iter * r
                if r:
                    sync.wait_ge(sem_l, base)
                sync.dma_start(left_t[:], left_in[:]).then_inc(sem_l, 16)
                sync.wait_ge(sem_l, base + 16)
                for ld in range(DLOC):
                    sync.dma_start(
                        out_l[:, ld, :], left_t[:]
                    ).then_inc(sem_l, 16)
            sync.wait_ge(sem_l, per_iter * repeat)

        @block.scalar
        def _(scalar):
            for r in range(repeat):
                base = per_iter * r
                if r:
                    scalar.wait_ge(sem_r, base)
                scalar.dma_start(rext_t[:], rext_in[:]).then_inc(sem_r, 16)
                scalar.wait_ge(sem_r, base + 16)
                for ld in range(DLOC):
                    scalar.dma_start(
                        out_r[:, ld, :], rext_t[:, ld:ld + SLAB]
                    ).then_inc(sem_r, 16)
            scalar.wait_ge(sem_r, per_iter * repeat)

    _NC_CACHE[repeat] = nc
    return nc


def _host_inputs(left, right):
    """Per-core device input dicts (host-side shard prep)."""
    le_flat = np.ascontiguousarray(left.reshape(B * C, HW))
    rf = right.reshape(B * C, H, W)

    in_maps = []
    for k in range(NCORES):
        d0 = DLOC * k
        re = np.zeros((B * C, H, PW), np.float32)
        take = max(0, W - d0)
        re[:, :, :take] = rf[:, :, d0:d0 + take]
        re_flat = np.zeros((B * C, SRCW), np.float32)
        re_flat[:, :SLAB] = re.reshape(B * C, SLAB)
        in_maps.append({"left": le_flat, "rext": re_flat})
    return in_maps


def _run(in_maps, **kwargs):
    nc = _build()
    return run_bass_kernel_spmd(nc, in_maps, list(range(NCORES)), **kwargs)


def _gather(results):
    out = np.empty((B, 2 * C, D, H, W), np.float32)
    for k in range(NCORES):
        dsl = slice(DLOC * k, DLOC * (k + 1))
        out[:, :C, dsl] = results[k]["out_l"].reshape(B, C, DLOC, H, W)
        slab_r = results[k]["out_r"].reshape(B, C, DLOC, H, PW)
        out[:, C:, dsl] = slab_r[:, :, :, :, :W]
    return out


def kernel(left_features, right_features, max_disparity):
    left = np.asarray(left_features, dtype=np.float32)
    right = np.asarray(right_features, dtype=np.float32)
    assert int(np.asarray(max_disparity)) == 4 * D
    assert left.shape == (B, C, H, W) and right.shape == (B, C, H, W)

    in_maps = _host_inputs(left, right)
    res = _run(in_maps)
    return _gather(res.results)
